# revision 97
# baseline (speedup 1.0000x reference)
"""Trainium2 Bass kernel for nn_LocalAttention (depthwise causal conv + RoPE +
windowed local attention), data-parallel over the batch dim on 8 NeuronCores.

Self-contained: hardcodes shapes B=32, N=4096, D=64, WS=128 and the sharding
(4 batches per core). Host-side prep is limited to dtype casts and weight
layout replication (Toeplitz band matrices from the depthwise conv weights,
RoPE cos/sin tables, which are pure indexing transforms); all FLOPs over the
activations run on device.
"""

import sys

sys.path.insert(0, "/opt/trn_rl_repo")

import ml_dtypes
import numpy as np

import concourse.bass as bass
import concourse.mybir as mybir
import concourse.tile as tile
import concourse.bass_utils as _bass_utils
from concourse.bass_utils import run_bass_kernel_spmd
from concourse.masks import make_identity



BF16 = mybir.dt.bfloat16
F32 = mybir.dt.float32
NPBF = ml_dtypes.bfloat16

B, N, D, WS = 32, 4096, 64, 128
W = N // WS              # 32 windows
NCORES = 8
BL = B // NCORES         # 4 batches per core
NWP = W + 1              # 33 window slots (slot 0 = zero pad = "window -1")
SCALE = D ** -0.5
XCOLS = BL * NWP * D     # 8448
QCOLS = D * BL * W       # 8192  (d, b, w)
PCOLS = W * 2 * WS       # 8192  (m, 256)
ROPE_BASE = 10000.0


def _split_multiwaits(nc, max_waits=1):
    """walrus in this env rejects >1 sem wait per instruction; split extras
    into standalone NoOp waits inserted just before, on the same engine."""
    n_fixed = 0
    for fn in nc.m.functions:
        for bb in fn.blocks:
            insts = bb.instructions
            new_list = []
            changed = False
            for inst in insts:
                si = inst.sync_info
                if si is not None and si.on_wait and len(si.on_wait) > max_waits:
                    waits = list(si.on_wait)
                    for w in waits[:-max_waits]:
                        nop = mybir.InstNoOp(
                            name=f"{inst.name}-xw{n_fixed}",
                            engine=inst.engine,
                            ins=[],
                            outs=[],
                            sync_info=mybir.SyncInfo(on_wait=[w], on_update=[]),
                        )
                        new_list.append(nop)
                        n_fixed += 1
                    si.on_wait = waits[-max_waits:]
                    changed = True
                new_list.append(inst)
            if changed:
                bb.instructions = new_list
    return n_fixed


def _ap(t, offset, dims):
    """AP over tile/dram tensor t: partition dim kept, free dims replaced."""
    return bass.AP(tensor=t.tensor, offset=t.offset + offset, ap=[t.ap[0]] + dims)


def _build_program():
    nc = bass.Bass()
    # host pre-tiled to the SBUF layout [128 j, (w, b, d)]
    xq = nc.dram_tensor("xq", [128, W * BL * D], BF16, kind="ExternalInput")
    xk = nc.dram_tensor("xk", [128, W * BL * D], BF16, kind="ExternalInput")
    xv = nc.dram_tensor("xv", [128, W * BL * D], BF16, kind="ExternalInput")
    # reversed-padded conv weights [d, 2 halves, 384]; the Toeplitz band
    # matrices are materialized by the DMA access pattern (diagonal-constant)
    tq = nc.dram_tensor("tq", [D, 2 * 384], BF16, kind="ExternalInput")
    tk = nc.dram_tensor("tk", [D, 2 * 384], BF16, kind="ExternalInput")
    tv = nc.dram_tensor("tv", [D, 2 * 384], BF16, kind="ExternalInput")
    cosb = nc.dram_tensor("cosb", [WS, D * W], BF16, kind="ExternalInput")
    sinb = nc.dram_tensor("sinb", [WS, D * W], BF16, kind="ExternalInput")
    # device layout [b, i, w, d]; host transposes back to [b, n, d]
    out = nc.dram_tensor("out", [BL, 128, W, D], F32, kind="ExternalOutput")

    xdram = {"q": xq, "k": xk, "v": xv}
    tdram = {"q": tq, "k": tk, "v": tv}

    with tile.TileContext(nc) as tc:
        import contextlib

        with contextlib.ExitStack() as ctx:
            const = ctx.enter_context(tc.tile_pool(name="const", bufs=1))
            xpool = ctx.enter_context(tc.tile_pool(name="x", bufs=3))
            tpool = ctx.enter_context(tc.tile_pool(name="toep", bufs=4))
            tabs = ctx.enter_context(tc.tile_pool(name="tabs", bufs=1))
            big = ctx.enter_context(tc.tile_pool(name="big", bufs=4))
            vpool = ctx.enter_context(tc.tile_pool(name="v", bufs=1))
            qtp = ctx.enter_context(tc.tile_pool(name="qt", bufs=4))
            opool = ctx.enter_context(tc.tile_pool(name="o", bufs=2))
            spool = ctx.enter_context(tc.tile_pool(name="s", bufs=2))

            # constants
            ident = const.tile([128, 128], BF16)
            make_identity(nc, ident)
            tri = const.tile([128, 128], BF16)  # tri[j,i] = 1 if i>=j else 0
            nc.vector.memset(tri[:], 1.0)
            nc.gpsimd.affine_select(
                out=tri[:], in_=tri[:], compare_op=mybir.AluOpType.is_ge,
                fill=0.0, base=0, channel_multiplier=-1, pattern=[[1, 128]],
            )
            ones = const.tile([128, 2], BF16)
            nc.vector.memset(ones[:], 1.0)

            # ---- load X tensors: [128 j, (w'=33, b, d)], w'=0 zeroed.
            # Host pre-tiled so each tensor is one contiguous-slab DMA.
            # xq goes first on sync; xk/xv SWDGE generation is delayed behind
            # tiny reads of the first Toeplitz chunks so the front of the
            # kernel isn't HBM-bandwidth starved.
            xt = {}
            for name in ("q", "k", "v"):
                t = xpool.tile([128, XCOLS], BF16, tag="x")
                nc.vector.memset(t[:, 0: BL * D], 0.0)
                xt[name] = t

            def load_x(name, eng):
                dst = _ap(xt[name], BL * D, [[1, W * BL * D]])
                eng.dma_start(out=dst, in_=xdram[name][:])

            # Toeplitz half-chunks: [128 j, (8 dloc, 2 h, 128 i)] per group g
            ttiles = {}

            def load_tt(name, g):
                # Both X and the Toeplitz tiles store the within-window
                # position axis reversed (p = 127-j), which makes the
                # diagonal-constant band expressible with all-positive
                # strides: tt[p, (dloc, h, i)] = R3[d, h, u] at u = p + i.
                tt = tpool.tile([128, 8 * 2 * 128], BF16, tag="toep")
                src = bass.AP(
                    tensor=tdram[name], offset=g * 8 * 768,
                    ap=[[1, 128], [384, 16], [1, 128]],
                )
                nc.sync.dma_start(out=tt[:], in_=src)
                ttiles[(name, g)] = tt

            # priority order: conv-q's first weights + xq get the HBM to
            # themselves; xk chains after xq and xv after xk via 1-element
            # WAW stubs (data deps the scheduler must honor)
            load_tt("q", 0)
            load_tt("q", 1)
            load_x("q", nc.sync)
            load_tt("q", 2)
            load_tt("q", 3)
            nc.vector.tensor_copy(xt["k"][0:1, BL * D: BL * D + 1],
                                  xt["q"][0:1, BL * D: BL * D + 1])
            load_x("k", nc.gpsimd)
            nc.vector.tensor_copy(xt["v"][0:1, BL * D: BL * D + 1],
                                  xt["k"][0:1, BL * D: BL * D + 1])
            load_x("v", nc.gpsimd)

            costab = tabs.tile([128, D * W], BF16)
            nc.scalar.dma_start(out=costab[:], in_=cosb[:])
            sintab = tabs.tile([128, D * W], BF16)
            nc.scalar.dma_start(out=sintab[:], in_=sinb[:])

            # rope/conv destination layout: [128 i, (b, d, w)]
            qc = big.tile([128, QCOLS], BF16, tag="big")
            kc = big.tile([128, QCOLS], BF16, tag="big")
            vsb = vpool.tile([128, BL * W * D], BF16)  # (b, d, w)

            qT = {}  # (tensor, pair) -> [128 (b2,d), (w,i)] bf16
            with tc.tile_pool(name="convps", bufs=2, space="PSUM") as convps, \
                 tc.tile_pool(name="tps", bufs=3, space="PSUM") as tps:
                # PE warmup: dependency-free matmuls at t~0 trip the HAM
                # un-throttle (1.2 -> 2.4 GHz) before the conv stream arrives
                with tc.tile_pool(name="warmps", bufs=1, space="PSUM") as wps:
                    wp = wps.tile([128, 128], F32)
                    for _ in range(72):
                        nc.tensor.matmul(wp[:], ident[:], ident[:],
                                         start=True, stop=True,
                                         skip_group_check=True)
                # ---- conv via per-channel Toeplitz matmuls
                drain_engs = (nc.scalar, nc.vector, nc.scalar, nc.vector,
                              nc.scalar, nc.vector, nc.scalar, nc.vector)

                def do_conv(name):
                    x = xt[name]
                    for g in range(8):  # 8 channels per psum group
                        if (name, g) not in ttiles:
                            load_tt(name, g)
                        for gp in (g + 1, g + 2):  # prefetch two ahead
                            if gp < 8 and (name, gp) not in ttiles:
                                load_tt(name, gp)
                        tt = ttiles[(name, g)]
                        cp = convps.tile([128, 8 * 128], F32)
                        for dd in range(8):
                            d = g * 8 + dd
                            lo = tt[:, (dd * 2) * 128: (dd * 2 + 1) * 128]
                            hi = tt[:, (dd * 2 + 1) * 128: (dd * 2 + 2) * 128]
                            # X is [j, (w'=33, b, d)]; rhs cols = (b, w)
                            rhs_lo = _ap(x, d, [[D, BL], [BL * D, W]])
                            rhs_hi = _ap(x, BL * D + d, [[D, BL], [BL * D, W]])
                            ps = cp[:, dd * 128: (dd + 1) * 128]
                            nc.tensor.matmul(ps, lo, rhs_lo, start=True, stop=False)
                            nc.tensor.matmul(ps, hi, rhs_hi, start=False, stop=True)
                        eng = drain_engs[g]
                        # cp is [128, (8 d, 4 b, 32 w)] -> (b, d, w) layout
                        dstt = qc if name == "q" else (kc if name == "k" else vsb)
                        src = _ap(cp, 0, [[128, 8], [32, BL], [1, W]])
                        dst = _ap(dstt, g * 8 * W, [[W, 8], [D * W, BL], [1, W]])
                        if eng is nc.scalar:
                            eng.copy(dst, src)
                        else:
                            eng.tensor_copy(dst, src)

                do_conv("q")
                do_conv("k")

                # ---- RoPE (DVE): x*cos + partner(x)*sin; layout (b, d, w)
                for name in ("q", "k"):
                    dstt = qc if name == "q" else kc
                    eng = nc.vector
                    t1 = big.tile([128, QCOLS], BF16, tag="big")
                    t2 = big.tile([128, QCOLS], BF16, tag="big")
                    # T1 = x * cos  (dims: (b, d, w))
                    cos_in = _ap(costab, 0, [[0, BL], [W, D], [1, W]])
                    x3 = _ap(dstt, 0, [[D * W, BL], [W, D], [1, W]])
                    t1v = _ap(t1, 0, [[D * W, BL], [W, D], [1, W]])
                    eng.tensor_mul(t1v, x3, cos_in)
                    # T2 = partner(x) * sin_signed (dims: (b, d2, pair, w))
                    part_in = _ap(dstt, W, [[D * W, BL], [2 * W, D // 2], [-W, 2], [1, W]])
                    sin_in = _ap(sintab, 0, [[0, BL], [2 * W, D // 2], [W, 2], [1, W]])
                    t2v = _ap(t2, 0, [[D * W, BL], [2 * W, D // 2], [W, 2], [1, W]])
                    eng.tensor_mul(t2v, part_in, sin_in)
                    # T3: x = t1 + t2
                    eng.tensor_add(dstt[:], t1[:], t2[:])

                # v conv now: PE runs it while the transposes below wait on
                # rope's DVE chain
                do_conv("v")

                # ---- transposes to [ (b2, d), (w, i) ] per (tensor, batch-pair)
                for name in ("q", "k"):
                    srct = qc if name == "q" else kc
                    for pair in range(2):
                        qt = qtp.tile([128, W * WS], BF16, tag="qt")
                        for w4 in range(8):
                            tp = tps.tile([128, 512], BF16)
                            for wi in range(4):
                                w = w4 * 4 + wi
                                # qc cols (b2 in pair, all d) at window w:
                                # [[D*W, 2], [W, 64]] collapses to [[W, 128]]
                                src = _ap(srct, pair * 2 * D * W + w,
                                          [[D * W, 2], [W, D]])
                                nc.tensor.transpose(
                                    tp[:, wi * 128: (wi + 1) * 128], src, ident[:]
                                )
                            dst = qt[:, w4 * 512: (w4 + 1) * 512]
                            if w4 % 2 == 0:
                                nc.vector.tensor_copy(dst, tp[:])
                            else:
                                nc.scalar.copy(dst, tp[:])
                        qT[(name, pair)] = qt

            # ---- attention per local batch
            with tc.tile_pool(name="simps", bufs=3, space="PSUM") as simps, \
                 tc.tile_pool(name="avps", bufs=1, space="PSUM") as avps:
                for b in range(BL):
                    pair, half = b // 2, b % 2
                    bp = half * 64
                    qt = qT[("q", pair)]
                    kt = qT[("k", pair)]
                    p = big.tile([128, PCOLS], BF16, tag="big")
                    # software-pipelined: sim/exp passes of 2 windows; after
                    # every 4 passes mask the 8-window group and immediately
                    # emit its AV matmuls so ACT (exp) and PE stay overlapped
                    av = avps.tile([128, W * D + W], F32)
                    HW2 = W // 2

                    def emit_av(w):
                        own = p[:, w * 256: w * 256 + 128]
                        ov = av[:, w * D: (w + 1) * D]
                        sv = av[:, W * D + w: W * D + w + 1]
                        vw = _ap(vsb, b * D * W + w, [[W, D]])
                        first_only = w == 0
                        nc.tensor.matmul(ov, own, vw, start=True,
                                         stop=first_only)
                        nc.tensor.matmul(sv, own, ones[:, 0:1], start=True,
                                         stop=first_only,
                                         skip_group_check=True)
                        if w > 0:
                            prev = p[:, (w - 1) * 256 + 128: w * 256]
                            vprev = _ap(vsb, b * D * W + w - 1, [[W, D]])
                            nc.tensor.matmul(ov, prev, vprev, start=False,
                                             stop=True, skip_group_check=True)
                            nc.tensor.matmul(sv, prev, ones[:, 0:1],
                                             start=False, stop=True,
                                             skip_group_check=True)

                    def emit_half_out(h):
                        sr = spool.tile([128, HW2], F32, tag="sr")
                        nc.vector.reciprocal(
                            sr[:],
                            av[:, W * D + h * HW2: W * D + (h + 1) * HW2])
                        ot = opool.tile([128, HW2 * D], F32, tag="ot")
                        # ot[(w, d)] = av[(w, d)] * sr[w] (broadcast over d)
                        av_v = _ap(av, h * HW2 * D, [[D, HW2], [1, D]])
                        sr_b = _ap(sr, 0, [[1, HW2], [0, D]])
                        ot_v = _ap(ot, 0, [[D, HW2], [1, D]])
                        nc.vector.tensor_mul(ot_v, av_v, sr_b)
                        # out_dev [b, i, w, d]: contiguous 4KB per partition
                        dstd = bass.AP(
                            tensor=out, offset=b * N * D + h * HW2 * D,
                            ap=[[W * D, 128], [1, HW2 * D]],
                        )
                        nc.sync.dma_start(out=dstd,
                                          in_=ot[:, 0: HW2 * D])

                    for t2 in range(16):
                        sp = simps.tile([128, 512], F32)
                        ncols_t = 0
                        for mi in range(2):
                            m = t2 * 2 + mi
                            ncols = 256 if m < W - 1 else 128
                            nc.tensor.matmul(
                                sp[:, mi * 256: mi * 256 + ncols],
                                kt[bp: bp + 64, m * 128: (m + 1) * 128],
                                qt[bp: bp + 64, m * 128: m * 128 + ncols],
                                start=True, stop=True,
                            )
                            ncols_t += ncols
                        nc.scalar.activation(
                            p[:, t2 * 512: t2 * 512 + ncols_t],
                            sp[:, :ncols_t],
                            mybir.ActivationFunctionType.Exp,
                        )
                        if t2 % 4 == 3:
                            # causal mask on own-halves of the last 8 windows
                            g8 = t2 // 4
                            pview = _ap(p, g8 * 8 * 256, [[256, 8], [1, 128]])
                            tri_b = _ap(tri, 0, [[0, 8], [1, 128]])
                            nc.vector.tensor_mul(pview, pview, tri_b)
                            if g8 == 0:
                                # pad-row fixup (key position 0 masked) and
                                # window-0 query-0 uniform-attention fixup
                                nc.vector.memset(p[0:1, 0:256], 0.0)
                                nc.vector.memset(p[:, 0:1], 1.0)
                            for w in range(g8 * 8, g8 * 8 + 8):
                                emit_av(w)
                            if g8 == 0:
                                # window-0 query-0 sum fix (+128 phantom keys)
                                nc.vector.tensor_scalar_add(
                                    av[0:1, W * D: W * D + 1],
                                    av[0:1, W * D: W * D + 1], 128.0,
                                )
                            if g8 == 1:
                                emit_half_out(0)
                            elif g8 == 3:
                                emit_half_out(1)

    _split_multiwaits(nc)
    return nc


_PROG = None


def _get_prog():
    global _PROG
    if _PROG is None:
        _PROG = _build_program()
    return _PROG


def _host_prep(q, k, v, wq, wk, wv):
    """Build per-core input maps (bf16 casts + constant tables)."""
    # Toeplitz bands [d, 2, j, i]: half0 (prev window): w[d, j-i-1];
    # half1 (own window): w[d, j-i+127]
    jj = np.arange(WS)[:, None]
    ii = np.arange(WS)[None, :]
    lod = jj - ii - 1
    hid = jj - ii + (WS - 1)
    lom = (lod >= 0) & (lod < WS)
    him = (hid >= 0) & (hid < WS)
    lodc = np.clip(lod, 0, WS - 1)
    hidc = np.clip(hid, 0, WS - 1)

    def toep(w, scale=1.0):
        # reversed-padded weights, duplicated per Toeplitz half:
        # R[u] = w[254-u] for u in [127, 254]; h=1 block = R, h=0 = R
        # shifted left by 128 (so addr u=127-j+i reads w[j-i-1+128h])
        wd = np.asarray(w, np.float32).reshape(D, WS) * scale
        r = np.zeros((D, 384), np.float32)
        r[:, 127:255] = wd[:, ::-1]
        r3 = np.zeros((D, 2, 384), np.float32)
        r3[:, 1, :] = r
        r3[:, 0, 0:256] = r[:, 128:384]
        return r3.reshape(D, 2 * 384).astype(NPBF)

    tq_np = toep(wq, SCALE)
    tk_np = toep(wk)
    tv_np = toep(wv)

    theta = 1.0 / ROPE_BASE ** (np.arange(0, D, 2, dtype=np.float32) / D)
    pm = np.arange(N, dtype=np.float32)[:, None] * theta[None, :]
    cos = np.repeat(np.cos(pm), 2, axis=-1)  # [n, d]
    sin = np.repeat(np.sin(pm), 2, axis=-1)
    sgn = np.where(np.arange(D) % 2 == 0, -1.0, 1.0).astype(np.float32)
    # [i, (d, w)] layout
    cosb_np = np.ascontiguousarray(
        cos.reshape(W, WS, D).transpose(1, 2, 0).reshape(WS, D * W)
    ).astype(NPBF)
    sinb_np = np.ascontiguousarray(
        (sin * sgn[None, :]).reshape(W, WS, D).transpose(1, 2, 0).reshape(WS, D * W)
    ).astype(NPBF)

    def tile_x(arr):
        # [BL, N, D] -> [128 p, (w, b, d)] contiguous, with the
        # within-window position axis reversed (p = 127-j) to match the
        # reversed-j Toeplitz tiles (contraction order is irrelevant)
        return np.ascontiguousarray(
            arr.reshape(BL, W, WS, D).transpose(2, 1, 0, 3)[::-1]
        ).reshape(WS, W * BL * D)

    qb = np.asarray(q, np.float32).astype(NPBF)
    kb = np.asarray(k, np.float32).astype(NPBF)
    vb = np.asarray(v, np.float32).astype(NPBF)

    in_maps = []
    for c in range(NCORES):
        sl = slice(c * BL, (c + 1) * BL)
        in_maps.append({
            "xq": tile_x(qb[sl]),
            "xk": tile_x(kb[sl]),
            "xv": tile_x(vb[sl]),
            "tq": tq_np, "tk": tk_np, "tv": tv_np,
            "cosb": cosb_np, "sinb": sinb_np,
        })
    return in_maps


def _install_ntff_hook():
    """Provide antenv.axon_hooks with a ctypes NTFF profile hook (the slim
    container lacks it); enables trace=True under axon."""
    import sys as _sys
    import types
    import ctypes
    import contextlib

    try:
        from antenv.axon_hooks import get_axon_ntff_profile_hook  # noqa: F401
        return
    except ImportError:
        pass
    so_path = "/opt/axon/libaxon_pjrt.so"
    try:
        lib = ctypes.CDLL(so_path)
    except OSError:
        return
    if not hasattr(lib, "axon_start_nrt_profile"):
        return
    lib.axon_start_nrt_profile.argtypes = [
        ctypes.POINTER(ctypes.c_int64), ctypes.c_size_t]
    lib.axon_start_nrt_profile.restype = ctypes.c_int64
    lib.axon_stop_nrt_profile.argtypes = [ctypes.c_char_p]
    lib.axon_stop_nrt_profile.restype = ctypes.c_int64

    @contextlib.contextmanager
    def _hook(output_dir, device_ids):
        import jax
        jax.devices()
        if device_ids:
            ids = (ctypes.c_int64 * len(device_ids))(*device_ids)
            rc = lib.axon_start_nrt_profile(ids, len(device_ids))
        else:
            rc = lib.axon_start_nrt_profile(None, 0)
        if rc != 0:
            raise RuntimeError(f"axon_start_nrt_profile rc={rc}")
        try:
            yield
        finally:
            n = lib.axon_stop_nrt_profile(str(output_dir).encode())
            print(f"profile: {n} file(s) written to {output_dir}")

    import antenv

    mod = types.ModuleType("antenv.axon_hooks")
    _state = {"hook": _hook}
    mod.set_axon_ntff_profile_hook = lambda h: _state.__setitem__("hook", h)
    mod.get_axon_ntff_profile_hook = lambda: _state["hook"]
    _sys.modules["antenv.axon_hooks"] = mod
    antenv.axon_hooks = mod


def run(q, k, v, wq, wk, wv, trace=False):
    nc = _get_prog()
    in_maps = _host_prep(q, k, v, wq, wk, wv)
    if trace:
        _install_ntff_hook()
    res = run_bass_kernel_spmd(nc, in_maps, core_ids=list(range(NCORES)),
                               trace=trace)
    # device out layout [b, i, w, d] -> [b, n=(w,i), d]
    outp = np.concatenate(
        [np.asarray(res.results[c]["out"]).reshape(BL, WS, W, D)
         .transpose(0, 2, 1, 3).reshape(BL, N, D)
         for c in range(NCORES)], axis=0)
    return outp, res


def kernel(q, k, v, wq, wk, wv):
    outp, _ = run(q, k, v, wq, wk, wv)
    return outp



# revision 98
# speedup vs baseline: 1.1106x; 1.1106x over previous
"""Trainium2 Bass kernel for nn_LocalAttention (depthwise causal conv + RoPE +
windowed local attention), data-parallel over the batch dim on 8 NeuronCores.

Self-contained: hardcodes shapes B=32, N=4096, D=64, WS=128 and the sharding
(4 batches per core). Host-side prep is limited to dtype casts and weight
layout replication (Toeplitz band matrices from the depthwise conv weights,
RoPE cos/sin tables, which are pure indexing transforms); all FLOPs over the
activations run on device.
"""

import sys

sys.path.insert(0, "/opt/trn_rl_repo")

import ml_dtypes
import numpy as np

import concourse.bass as bass
import concourse.mybir as mybir
import concourse.tile as tile
import concourse.bass_utils as _bass_utils
from concourse.bass_utils import run_bass_kernel_spmd
from concourse.masks import make_identity



BF16 = mybir.dt.bfloat16
F32 = mybir.dt.float32
NPBF = ml_dtypes.bfloat16

B, N, D, WS = 32, 4096, 64, 128
W = N // WS              # 32 windows
NCORES = 8
BL = B // NCORES         # 4 batches per core
NWP = W + 1              # 33 window slots (slot 0 = zero pad = "window -1")
SCALE = D ** -0.5
XCOLS = BL * NWP * D     # 8448
QCOLS = D * BL * W       # 8192  (d, b, w)
PCOLS = W * 2 * WS       # 8192  (m, 256)
ROPE_BASE = 10000.0


def _split_multiwaits(nc, max_waits=1):
    """walrus in this env rejects >1 sem wait per instruction; split extras
    into standalone NoOp waits inserted just before, on the same engine."""
    n_fixed = 0
    for fn in nc.m.functions:
        for bb in fn.blocks:
            insts = bb.instructions
            new_list = []
            changed = False
            for inst in insts:
                si = inst.sync_info
                if si is not None and si.on_wait and len(si.on_wait) > max_waits:
                    waits = list(si.on_wait)
                    for w in waits[:-max_waits]:
                        nop = mybir.InstNoOp(
                            name=f"{inst.name}-xw{n_fixed}",
                            engine=inst.engine,
                            ins=[],
                            outs=[],
                            sync_info=mybir.SyncInfo(on_wait=[w], on_update=[]),
                        )
                        new_list.append(nop)
                        n_fixed += 1
                    si.on_wait = waits[-max_waits:]
                    changed = True
                new_list.append(inst)
            if changed:
                bb.instructions = new_list
    return n_fixed


def _ap(t, offset, dims):
    """AP over tile/dram tensor t: partition dim kept, free dims replaced."""
    return bass.AP(tensor=t.tensor, offset=t.offset + offset, ap=[t.ap[0]] + dims)


def _build_program():
    nc = bass.Bass()
    # host pre-tiled to the SBUF layout [128 j, (w, b, d)]
    xq = nc.dram_tensor("xq", [128, W * BL * D], BF16, kind="ExternalInput")
    xk = nc.dram_tensor("xk", [128, W * BL * D], BF16, kind="ExternalInput")
    xv = nc.dram_tensor("xv", [128, W * BL * D], BF16, kind="ExternalInput")
    # reversed-padded conv weights [d, 2 halves, 384]; the Toeplitz band
    # matrices are materialized by the DMA access pattern (diagonal-constant)
    tq = nc.dram_tensor("tq", [D, 2 * 384], BF16, kind="ExternalInput")
    tk = nc.dram_tensor("tk", [D, 2 * 384], BF16, kind="ExternalInput")
    tv = nc.dram_tensor("tv", [D, 2 * 384], BF16, kind="ExternalInput")
    cosb = nc.dram_tensor("cosb", [WS, D * W], BF16, kind="ExternalInput")
    sinb = nc.dram_tensor("sinb", [WS, D * W], BF16, kind="ExternalInput")
    # device layout [b, i, w, d]; host transposes back to [b, n, d]
    out = nc.dram_tensor("out", [BL, 128, W, D], F32, kind="ExternalOutput")

    xdram = {"q": xq, "k": xk, "v": xv}
    tdram = {"q": tq, "k": tk, "v": tv}

    with tile.TileContext(nc) as tc:
        import contextlib

        with contextlib.ExitStack() as ctx:
            const = ctx.enter_context(tc.tile_pool(name="const", bufs=1))
            xpool = ctx.enter_context(tc.tile_pool(name="x", bufs=3))
            tpool = ctx.enter_context(tc.tile_pool(name="toep", bufs=4))
            tabs = ctx.enter_context(tc.tile_pool(name="tabs", bufs=1))
            big = ctx.enter_context(tc.tile_pool(name="big", bufs=4))
            vpool = ctx.enter_context(tc.tile_pool(name="v", bufs=1))
            qtp = ctx.enter_context(tc.tile_pool(name="qt", bufs=4))
            opool = ctx.enter_context(tc.tile_pool(name="o", bufs=2))
            spool = ctx.enter_context(tc.tile_pool(name="s", bufs=2))

            # constants
            ident = const.tile([128, 128], BF16)
            make_identity(nc, ident)
            tri = const.tile([128, 128], BF16)  # tri[j,i] = 1 if i>=j else 0
            nc.vector.memset(tri[:], 1.0)
            nc.gpsimd.affine_select(
                out=tri[:], in_=tri[:], compare_op=mybir.AluOpType.is_ge,
                fill=0.0, base=0, channel_multiplier=-1, pattern=[[1, 128]],
            )
            ones = const.tile([128, 2], BF16)
            nc.vector.memset(ones[:], 1.0)

            # ---- load X tensors: [128 j, (w'=33, b, d)], w'=0 zeroed.
            # Host pre-tiled so each tensor is one contiguous-slab DMA.
            # xq goes first on sync; xk/xv SWDGE generation is delayed behind
            # tiny reads of the first Toeplitz chunks so the front of the
            # kernel isn't HBM-bandwidth starved.
            xt = {}
            for name in ("q", "k", "v"):
                t = xpool.tile([128, XCOLS], BF16, tag="x")
                nc.vector.memset(t[:, 0: BL * D], 0.0)
                xt[name] = t

            def load_x(name, eng):
                dst = _ap(xt[name], BL * D, [[1, W * BL * D]])
                eng.dma_start(out=dst, in_=xdram[name][:])

            # Toeplitz half-chunks: [128 j, (8 dloc, 2 h, 128 i)] per group g
            ttiles = {}

            def load_tt(name, g):
                # Both X and the Toeplitz tiles store the within-window
                # position axis reversed (p = 127-j), which makes the
                # diagonal-constant band expressible with all-positive
                # strides: tt[p, (dloc, h, i)] = R3[d, h, u] at u = p + i.
                tt = tpool.tile([128, 8 * 2 * 128], BF16, tag="toep")
                src = bass.AP(
                    tensor=tdram[name], offset=g * 8 * 768,
                    ap=[[1, 128], [384, 16], [1, 128]],
                )
                nc.sync.dma_start(out=tt[:], in_=src)
                ttiles[(name, g)] = tt

            # priority order: conv-q's first weights + xq get the HBM to
            # themselves; xk chains after xq and xv after xk via 1-element
            # WAW stubs (data deps the scheduler must honor)
            load_tt("q", 0)
            load_tt("q", 1)
            load_x("q", nc.sync)
            load_tt("q", 2)
            load_tt("q", 3)
            nc.vector.tensor_copy(xt["k"][0:1, BL * D: BL * D + 1],
                                  xt["q"][0:1, BL * D: BL * D + 1])
            load_x("k", nc.gpsimd)
            nc.vector.tensor_copy(xt["v"][0:1, BL * D: BL * D + 1],
                                  xt["k"][0:1, BL * D: BL * D + 1])
            load_x("v", nc.gpsimd)

            costab = tabs.tile([128, D * W], BF16)
            nc.scalar.dma_start(out=costab[:], in_=cosb[:])
            sintab = tabs.tile([128, D * W], BF16)
            nc.scalar.dma_start(out=sintab[:], in_=sinb[:])

            # rope/conv destination layout: [128 i, (b, d, w)]
            qc = big.tile([128, QCOLS], BF16, tag="big")
            kc = big.tile([128, QCOLS], BF16, tag="big")
            vsb = vpool.tile([128, BL * W * D], BF16)  # (b, d, w)

            qT = {}  # (tensor, pair) -> [128 (b2,d), (w,i)] bf16
            with tc.tile_pool(name="convps", bufs=2, space="PSUM") as convps, \
                 tc.tile_pool(name="tps", bufs=3, space="PSUM") as tps:
                # PE warmup: dependency-free matmuls at t~0 trip the HAM
                # un-throttle (1.2 -> 2.4 GHz) before the conv stream arrives
                with tc.tile_pool(name="warmps", bufs=1, space="PSUM") as wps:
                    wp = wps.tile([128, 128], F32)
                    for _ in range(72):
                        nc.tensor.matmul(wp[:], ident[:], ident[:],
                                         start=True, stop=True,
                                         skip_group_check=True)
                # ---- conv via per-channel Toeplitz matmuls
                drain_engs = (nc.scalar, nc.vector, nc.scalar, nc.vector,
                              nc.scalar, nc.vector, nc.scalar, nc.vector)

                def do_conv(name):
                    x = xt[name]
                    for g in range(8):  # 8 channels per psum group
                        if (name, g) not in ttiles:
                            load_tt(name, g)
                        if g < 7 and (name, g + 1) not in ttiles:
                            load_tt(name, g + 1)  # prefetch
                        tt = ttiles[(name, g)]
                        cp = convps.tile([128, 8 * 128], F32)
                        for dd in range(8):
                            d = g * 8 + dd
                            lo = tt[:, (dd * 2) * 128: (dd * 2 + 1) * 128]
                            hi = tt[:, (dd * 2 + 1) * 128: (dd * 2 + 2) * 128]
                            # X is [j, (w'=33, b, d)]; rhs cols = (b, w)
                            rhs_lo = _ap(x, d, [[D, BL], [BL * D, W]])
                            rhs_hi = _ap(x, BL * D + d, [[D, BL], [BL * D, W]])
                            ps = cp[:, dd * 128: (dd + 1) * 128]
                            nc.tensor.matmul(ps, lo, rhs_lo, start=True, stop=False)
                            nc.tensor.matmul(ps, hi, rhs_hi, start=False, stop=True)
                        eng = drain_engs[g]
                        # cp is [128, (8 d, 4 b, 32 w)] -> (b, d, w) layout
                        dstt = qc if name == "q" else (kc if name == "k" else vsb)
                        src = _ap(cp, 0, [[128, 8], [32, BL], [1, W]])
                        dst = _ap(dstt, g * 8 * W, [[W, 8], [D * W, BL], [1, W]])
                        if eng is nc.scalar:
                            eng.copy(dst, src)
                        else:
                            eng.tensor_copy(dst, src)

                do_conv("q")
                do_conv("k")

                # ---- RoPE (DVE): x*cos + partner(x)*sin; layout (b, d, w)
                for name in ("q", "k"):
                    dstt = qc if name == "q" else kc
                    eng = nc.vector
                    t1 = big.tile([128, QCOLS], BF16, tag="big")
                    t2 = big.tile([128, QCOLS], BF16, tag="big")
                    # T1 = x * cos  (dims: (b, d, w))
                    cos_in = _ap(costab, 0, [[0, BL], [W, D], [1, W]])
                    x3 = _ap(dstt, 0, [[D * W, BL], [W, D], [1, W]])
                    t1v = _ap(t1, 0, [[D * W, BL], [W, D], [1, W]])
                    eng.tensor_mul(t1v, x3, cos_in)
                    # T2 = partner(x) * sin_signed (dims: (b, d2, pair, w))
                    part_in = _ap(dstt, W, [[D * W, BL], [2 * W, D // 2], [-W, 2], [1, W]])
                    sin_in = _ap(sintab, 0, [[0, BL], [2 * W, D // 2], [W, 2], [1, W]])
                    t2v = _ap(t2, 0, [[D * W, BL], [2 * W, D // 2], [W, 2], [1, W]])
                    eng.tensor_mul(t2v, part_in, sin_in)
                    # T3: x = t1 + t2
                    eng.tensor_add(dstt[:], t1[:], t2[:])

                # v conv now: PE runs it while the transposes below wait on
                # rope's DVE chain
                do_conv("v")

                # ---- transposes to [ (b2, d), (w, i) ] per (tensor, batch-pair)
                for name in ("q", "k"):
                    srct = qc if name == "q" else kc
                    for pair in range(2):
                        qt = qtp.tile([128, W * WS], BF16, tag="qt")
                        for w4 in range(8):
                            tp = tps.tile([128, 512], BF16)
                            for wi in range(4):
                                w = w4 * 4 + wi
                                # qc cols (b2 in pair, all d) at window w:
                                # [[D*W, 2], [W, 64]] collapses to [[W, 128]]
                                src = _ap(srct, pair * 2 * D * W + w,
                                          [[D * W, 2], [W, D]])
                                nc.tensor.transpose(
                                    tp[:, wi * 128: (wi + 1) * 128], src, ident[:]
                                )
                            dst = qt[:, w4 * 512: (w4 + 1) * 512]
                            if w4 % 2 == 0:
                                nc.vector.tensor_copy(dst, tp[:])
                            else:
                                nc.scalar.copy(dst, tp[:])
                        qT[(name, pair)] = qt

            # ---- attention per local batch
            with tc.tile_pool(name="simps", bufs=3, space="PSUM") as simps, \
                 tc.tile_pool(name="avps", bufs=1, space="PSUM") as avps:
                for b in range(BL):
                    pair, half = b // 2, b % 2
                    bp = half * 64
                    qt = qT[("q", pair)]
                    kt = qT[("k", pair)]
                    p = big.tile([128, PCOLS], BF16, tag="big")
                    # software-pipelined: sim/exp passes of 2 windows; after
                    # every 4 passes mask the 8-window group and immediately
                    # emit its AV matmuls so ACT (exp) and PE stay overlapped
                    av = avps.tile([128, W * D + W], F32)
                    HW2 = W // 2

                    def emit_av(w):
                        own = p[:, w * 256: w * 256 + 128]
                        ov = av[:, w * D: (w + 1) * D]
                        sv = av[:, W * D + w: W * D + w + 1]
                        vw = _ap(vsb, b * D * W + w, [[W, D]])
                        first_only = w == 0
                        nc.tensor.matmul(ov, own, vw, start=True,
                                         stop=first_only)
                        nc.tensor.matmul(sv, own, ones[:, 0:1], start=True,
                                         stop=first_only,
                                         skip_group_check=True)
                        if w > 0:
                            prev = p[:, (w - 1) * 256 + 128: w * 256]
                            vprev = _ap(vsb, b * D * W + w - 1, [[W, D]])
                            nc.tensor.matmul(ov, prev, vprev, start=False,
                                             stop=True, skip_group_check=True)
                            nc.tensor.matmul(sv, prev, ones[:, 0:1],
                                             start=False, stop=True,
                                             skip_group_check=True)

                    def emit_half_out(h):
                        sr = spool.tile([128, HW2], F32, tag="sr")
                        nc.vector.reciprocal(
                            sr[:],
                            av[:, W * D + h * HW2: W * D + (h + 1) * HW2])
                        ot = opool.tile([128, HW2 * D], F32, tag="ot")
                        # ot[(w, d)] = av[(w, d)] * sr[w] (broadcast over d)
                        av_v = _ap(av, h * HW2 * D, [[D, HW2], [1, D]])
                        sr_b = _ap(sr, 0, [[1, HW2], [0, D]])
                        ot_v = _ap(ot, 0, [[D, HW2], [1, D]])
                        nc.vector.tensor_mul(ot_v, av_v, sr_b)
                        # out_dev [b, i, w, d]: contiguous 4KB per partition
                        dstd = bass.AP(
                            tensor=out, offset=b * N * D + h * HW2 * D,
                            ap=[[W * D, 128], [1, HW2 * D]],
                        )
                        nc.sync.dma_start(out=dstd,
                                          in_=ot[:, 0: HW2 * D])

                    for t2 in range(16):
                        sp = simps.tile([128, 512], F32)
                        ncols_t = 0
                        for mi in range(2):
                            m = t2 * 2 + mi
                            ncols = 256 if m < W - 1 else 128
                            nc.tensor.matmul(
                                sp[:, mi * 256: mi * 256 + ncols],
                                kt[bp: bp + 64, m * 128: (m + 1) * 128],
                                qt[bp: bp + 64, m * 128: m * 128 + ncols],
                                start=True, stop=True,
                            )
                            ncols_t += ncols
                        nc.scalar.activation(
                            p[:, t2 * 512: t2 * 512 + ncols_t],
                            sp[:, :ncols_t],
                            mybir.ActivationFunctionType.Exp,
                        )
                        if t2 % 4 == 3:
                            # causal mask on own-halves of the last 8 windows
                            g8 = t2 // 4
                            pview = _ap(p, g8 * 8 * 256, [[256, 8], [1, 128]])
                            tri_b = _ap(tri, 0, [[0, 8], [1, 128]])
                            nc.vector.tensor_mul(pview, pview, tri_b)
                            if g8 == 0:
                                # pad-row fixup (key position 0 masked) and
                                # window-0 query-0 uniform-attention fixup
                                nc.vector.memset(p[0:1, 0:256], 0.0)
                                nc.vector.memset(p[:, 0:1], 1.0)
                            for w in range(g8 * 8, g8 * 8 + 8):
                                emit_av(w)
                            if g8 == 0:
                                # window-0 query-0 sum fix (+128 phantom keys)
                                nc.vector.tensor_scalar_add(
                                    av[0:1, W * D: W * D + 1],
                                    av[0:1, W * D: W * D + 1], 128.0,
                                )
                            if g8 == 1:
                                emit_half_out(0)
                            elif g8 == 3:
                                emit_half_out(1)

    _split_multiwaits(nc)
    return nc


_PROG = None


def _get_prog():
    global _PROG
    if _PROG is None:
        _PROG = _build_program()
    return _PROG


def _host_prep(q, k, v, wq, wk, wv):
    """Build per-core input maps (bf16 casts + constant tables)."""
    # Toeplitz bands [d, 2, j, i]: half0 (prev window): w[d, j-i-1];
    # half1 (own window): w[d, j-i+127]
    jj = np.arange(WS)[:, None]
    ii = np.arange(WS)[None, :]
    lod = jj - ii - 1
    hid = jj - ii + (WS - 1)
    lom = (lod >= 0) & (lod < WS)
    him = (hid >= 0) & (hid < WS)
    lodc = np.clip(lod, 0, WS - 1)
    hidc = np.clip(hid, 0, WS - 1)

    def toep(w, scale=1.0):
        # reversed-padded weights, duplicated per Toeplitz half:
        # R[u] = w[254-u] for u in [127, 254]; h=1 block = R, h=0 = R
        # shifted left by 128 (so addr u=127-j+i reads w[j-i-1+128h])
        wd = np.asarray(w, np.float32).reshape(D, WS) * scale
        r = np.zeros((D, 384), np.float32)
        r[:, 127:255] = wd[:, ::-1]
        r3 = np.zeros((D, 2, 384), np.float32)
        r3[:, 1, :] = r
        r3[:, 0, 0:256] = r[:, 128:384]
        return r3.reshape(D, 2 * 384).astype(NPBF)

    tq_np = toep(wq, SCALE)
    tk_np = toep(wk)
    tv_np = toep(wv)

    theta = 1.0 / ROPE_BASE ** (np.arange(0, D, 2, dtype=np.float32) / D)
    pm = np.arange(N, dtype=np.float32)[:, None] * theta[None, :]
    cos = np.repeat(np.cos(pm), 2, axis=-1)  # [n, d]
    sin = np.repeat(np.sin(pm), 2, axis=-1)
    sgn = np.where(np.arange(D) % 2 == 0, -1.0, 1.0).astype(np.float32)
    # [i, (d, w)] layout
    cosb_np = np.ascontiguousarray(
        cos.reshape(W, WS, D).transpose(1, 2, 0).reshape(WS, D * W)
    ).astype(NPBF)
    sinb_np = np.ascontiguousarray(
        (sin * sgn[None, :]).reshape(W, WS, D).transpose(1, 2, 0).reshape(WS, D * W)
    ).astype(NPBF)

    def tile_x(arr):
        # [BL, N, D] -> [128 p, (w, b, d)] contiguous, with the
        # within-window position axis reversed (p = 127-j) to match the
        # reversed-j Toeplitz tiles (contraction order is irrelevant)
        return np.ascontiguousarray(
            arr.reshape(BL, W, WS, D).transpose(2, 1, 0, 3)[::-1]
        ).reshape(WS, W * BL * D)

    qb = np.asarray(q, np.float32).astype(NPBF)
    kb = np.asarray(k, np.float32).astype(NPBF)
    vb = np.asarray(v, np.float32).astype(NPBF)

    in_maps = []
    for c in range(NCORES):
        sl = slice(c * BL, (c + 1) * BL)
        in_maps.append({
            "xq": tile_x(qb[sl]),
            "xk": tile_x(kb[sl]),
            "xv": tile_x(vb[sl]),
            "tq": tq_np, "tk": tk_np, "tv": tv_np,
            "cosb": cosb_np, "sinb": sinb_np,
        })
    return in_maps


def _install_ntff_hook():
    """Provide antenv.axon_hooks with a ctypes NTFF profile hook (the slim
    container lacks it); enables trace=True under axon."""
    import sys as _sys
    import types
    import ctypes
    import contextlib

    try:
        from antenv.axon_hooks import get_axon_ntff_profile_hook  # noqa: F401
        return
    except ImportError:
        pass
    so_path = "/opt/axon/libaxon_pjrt.so"
    try:
        lib = ctypes.CDLL(so_path)
    except OSError:
        return
    if not hasattr(lib, "axon_start_nrt_profile"):
        return
    lib.axon_start_nrt_profile.argtypes = [
        ctypes.POINTER(ctypes.c_int64), ctypes.c_size_t]
    lib.axon_start_nrt_profile.restype = ctypes.c_int64
    lib.axon_stop_nrt_profile.argtypes = [ctypes.c_char_p]
    lib.axon_stop_nrt_profile.restype = ctypes.c_int64

    @contextlib.contextmanager
    def _hook(output_dir, device_ids):
        import jax
        jax.devices()
        if device_ids:
            ids = (ctypes.c_int64 * len(device_ids))(*device_ids)
            rc = lib.axon_start_nrt_profile(ids, len(device_ids))
        else:
            rc = lib.axon_start_nrt_profile(None, 0)
        if rc != 0:
            raise RuntimeError(f"axon_start_nrt_profile rc={rc}")
        try:
            yield
        finally:
            n = lib.axon_stop_nrt_profile(str(output_dir).encode())
            print(f"profile: {n} file(s) written to {output_dir}")

    import antenv

    mod = types.ModuleType("antenv.axon_hooks")
    _state = {"hook": _hook}
    mod.set_axon_ntff_profile_hook = lambda h: _state.__setitem__("hook", h)
    mod.get_axon_ntff_profile_hook = lambda: _state["hook"]
    _sys.modules["antenv.axon_hooks"] = mod
    antenv.axon_hooks = mod


def run(q, k, v, wq, wk, wv, trace=False):
    nc = _get_prog()
    in_maps = _host_prep(q, k, v, wq, wk, wv)
    if trace:
        _install_ntff_hook()
    res = run_bass_kernel_spmd(nc, in_maps, core_ids=list(range(NCORES)),
                               trace=trace)
    # device out layout [b, i, w, d] -> [b, n=(w,i), d]
    outp = np.concatenate(
        [np.asarray(res.results[c]["out"]).reshape(BL, WS, W, D)
         .transpose(0, 2, 1, 3).reshape(BL, N, D)
         for c in range(NCORES)], axis=0)
    return outp, res


def kernel(q, k, v, wq, wk, wv):
    outp, _ = run(q, k, v, wq, wk, wv)
    return outp

